# revision 8
# baseline (speedup 1.0000x reference)
"""Trainium2 Bass kernel for nn_ContrastLoss3 (multi-positive contrastive loss).

Math (matches the reference):
  f = L2-normalize(input_f.reshape(N, D)) rows;  sim = f @ f.T  (N = B*T = 6912)
  per valid row i:  A_i = sum_{j valid, lab_j == lab_i} exp(-sim_ij)
                    B_i = sum_{j valid, lab_j != lab_i} exp(+sim_ij)
                    loss_i = log(1 + A_i * B_i)
  out = sum_i valid loss_i / n_valid

Device strategy (8 cores, row-sharded):
  - Pad N to 7168 = 56*128. Each core owns a 896-row block (i), computes the
    full sim column-block S[j, i] for all j via PE matmuls (bf16, fp32 PSUM),
    contracting D=1024 in 8 k-tiles of 128.
  - E = exp(S), R = exp(-S) on the scalar engine (ACT), bf16.
  - Masked sums become matmuls against a one-hot label matrix OHV [j, 96]
    (95 classes x valid + a valid column), accumulated in PSUM across all 56
    j-tiles:  G_E[c, i] = sum_j ohv[j,c] E[j,i], same for R.
  - Epilogue: A_i = sum_c ohr[c,i] G_R[c,i]; B_i = G_E[95,i] - sum_c ohr G_E;
    loss_i = ln(1 + A_i B_i); masked row-sum -> one scalar per core.
  - Normalization: per-row sum-of-squares via ACT Square+accum in row layout,
    1/sqrt via a Newton iteration on DVE (seed 1/32 since q ~= D), scale+cast
    to bf16 on DVE, then on-device DMA xbar transpose to the [d, j] layout
    the PE needs.
  Host only pads/reshapes/transposes-none, builds one-hot index encodings,
  and sums the 8 per-core partials / n_valid.
"""

import numpy as np
import ml_dtypes

# ---- geometry (hardcoded for this problem) ----
B, T, D = 256, 27, 1024
N_REAL = B * T            # 6912
NP_PAD = 7168             # 56 * 128
NCORES = 8
LOC = NP_PAD // NCORES    # 896 rows per core
JT = NP_PAD // 128        # 56 column tiles
LT = LOC // 128           # 7 local tiles
TN = JT + LT              # 63 row tiles streamed in (local first)
KT = D // 128             # 8 contraction tiles
NCLS = 95                 # labels are 1..95
MC = 96                   # ohv cols: 95 classes + valid column
H0, H1 = 512, 384         # matmul free-dim split of 896 (PSUM bank = 512 fp32)

_BUILT = None


def _build():
    import concourse.tile as tile
    from concourse import bacc, mybir

    f32 = mybir.dt.float32
    bf16 = mybir.dt.bfloat16
    AF = mybir.ActivationFunctionType
    OP = mybir.AluOpType

    nc = bacc.Bacc("TRN2", target_bir_lowering=False, debug=False)

    xa = nc.dram_tensor("xa", [TN, 128, D], f32, kind="ExternalInput")
    ohv = nc.dram_tensor("ohv", [128, JT, MC], bf16, kind="ExternalInput")
    ohr = nc.dram_tensor("ohr", [MC, LOC], f32, kind="ExternalInput")
    vloc = nc.dram_tensor("vloc", [1, LOC], f32, kind="ExternalInput")
    # reduction weight columns: wa = [1]*95+[0], wb = [-1]*95+[1]
    wa = nc.dram_tensor("wa", [MC, 1], f32, kind="ExternalInput")
    wb = nc.dram_tensor("wb", [MC, 1], f32, kind="ExternalInput")
    outp = nc.dram_tensor("out", [1, 1], f32, kind="ExternalOutput")

    with tile.TileContext(nc) as tc:
        with (
            tc.tile_pool(name="singles", bufs=1) as singles,
            tc.tile_pool(name="x32", bufs=3) as x32p,
            tc.tile_pool(name="xraw", bufs=10) as xrawp,
            tc.tile_pool(name="xn", bufs=3) as xnp_,
            tc.tile_pool(name="sq", bufs=2) as sqp,
            tc.tile_pool(name="newt", bufs=2) as ntp,
            tc.tile_pool(name="ebuf", bufs=2) as ep,
            tc.tile_pool(name="rbuf", bufs=2) as rp,
            tc.tile_pool(name="eps", bufs=2) as epsp,
        ):
            fT = singles.tile([128, KT, NP_PAD], bf16)     # [d_lo, d_hi, j]
            fLoc = singles.tile([128, KT, LOC], bf16)      # [d_lo, d_hi, i]
            ohv_sb = singles.tile([128, JT, MC], bf16)
            ohr_sb = singles.tile([MC, LOC], f32)
            vloc_sb = singles.tile([1, LOC], f32)
            sumsq = singles.tile([128, TN + 1], f32)
            s_all = singles.tile([128, TN + 1], f32)
            wa_sb = singles.tile([MC, 1], f32)
            wb_sb = singles.tile([MC, 1], f32)

            nc.sync.dma_start(ohv_sb[:], ohv[:, :, :])
            nc.sync.dma_start(ohr_sb[:], ohr[:, :])
            nc.sync.dma_start(vloc_sb[:], vloc[:, :])
            nc.sync.dma_start(wa_sb[:], wa[:, :])
            nc.sync.dma_start(wb_sb[:], wb[:, :])

            # ---- phase A: load, sumsq, rsqrt, normalize+cast, transpose ----
            # groups: tiles [0,7) = local block, then 7 groups of 8 full tiles
            groups = [list(range(0, LT))] + [
                list(range(LT + 8 * g, LT + min(8 * g + 8, JT))) for g in range(7)
            ]
            c0 = 1.0 / 32.0  # rsqrt seed: q ~= D = 1024

            for grp in groups:
                xraws = {}
                for t in grp:
                    x32 = x32p.tile([128, D], f32)
                    nc.sync.dma_start(x32[:], xa[t, :, :])
                    xraw = xrawp.tile([128, D], bf16)
                    nc.vector.tensor_scalar_mul(xraw[:], x32[:], 1.0)
                    sq = sqp.tile([128, D], bf16)
                    nc.scalar.activation(
                        sq[:], xraw[:], AF.Square,
                        accum_out=sumsq[:, t : t + 1],
                    )
                    xraws[t] = xraw
                a, b = grp[0], grp[-1] + 1
                q = sumsq[:, a:b]
                y = s_all[:, a:b]
                # y0 = c0*(1.5 - 0.5*c0^2*q) ; then Newton y <- y*(1.5 - 0.5*q*y^2)
                nc.vector.tensor_scalar(
                    out=y, in0=q, scalar1=-0.5 * c0**3, scalar2=1.5 * c0,
                    op0=OP.mult, op1=OP.add,
                )
                for _ in range(4):
                    t2 = ntp.tile([128, 8], f32)
                    w = b - a
                    nc.vector.tensor_mul(t2[:, :w], y, y)
                    nc.vector.tensor_mul(t2[:, :w], t2[:, :w], q)
                    nc.vector.tensor_scalar(
                        out=t2[:, :w], in0=t2[:, :w], scalar1=-0.5, scalar2=1.5,
                        op0=OP.mult, op1=OP.add,
                    )
                    nc.vector.tensor_mul(y, y, t2[:, :w])
                for t in grp:
                    xn = xnp_.tile([128, D], bf16)
                    nc.vector.tensor_scalar_mul(xn[:], xraws[t][:], s_all[:, t : t + 1])
                    if t < LT:
                        dest = fLoc[:, :, 128 * t : 128 * (t + 1)]
                    else:
                        tt = t - LT
                        dest = fT[:, :, 128 * tt : 128 * (tt + 1)]
                    nc.sync.dma_start_transpose(dest, xn[:])

            # ---- phases B+C: matmuls, exp, one-hot reductions, epilogue ----
            with tc.tile_pool(name="psG", bufs=1, space="PSUM") as psGp:
                psGE = psGp.tile([MC, 1024], f32)
                psGR = psGp.tile([MC, 1024], f32)

                with tc.tile_pool(name="psS", bufs=2, space="PSUM") as psSp:
                    for jt in range(JT):
                        psS = psSp.tile([128, 1024], f32)
                        for k in range(KT):
                            w = fT[:, k, 128 * jt : 128 * (jt + 1)]
                            nc.tensor.matmul(
                                psS[:, 0:H0], w, fLoc[:, k, 0:H0],
                                start=(k == 0), stop=(k == KT - 1),
                            )
                            nc.tensor.matmul(
                                psS[:, H0:LOC], w, fLoc[:, k, H0:LOC],
                                start=(k == 0), stop=(k == KT - 1),
                            )
                        e_t = ep.tile([128, LOC], bf16)
                        nc.scalar.activation(e_t[:], psS[:, 0:LOC], AF.Exp, scale=1.0)
                        r_t = rp.tile([128, LOC], bf16)
                        nc.scalar.activation(r_t[:], psS[:, 0:LOC], AF.Exp, scale=-1.0)
                        w2 = ohv_sb[:, jt, :]
                        st, sp = (jt == 0), (jt == JT - 1)
                        nc.tensor.matmul(psGE[:, 0:H0], w2, e_t[:, 0:H0],
                                         start=st, stop=sp, skip_group_check=True)
                        nc.tensor.matmul(psGE[:, H0:LOC], w2, e_t[:, H0:LOC],
                                         start=st, stop=sp, skip_group_check=True)
                        nc.tensor.matmul(psGR[:, 0:H0], w2, r_t[:, 0:H0],
                                         start=st, stop=sp, skip_group_check=True)
                        nc.tensor.matmul(psGR[:, H0:LOC], w2, r_t[:, H0:LOC],
                                         start=st, stop=sp, skip_group_check=True)

                # ---- epilogue ----
                # ohr rows 0..94: one-hot of local labels; row 95: all ones.
                # A = wa.T @ (G_R . ohr);  B = wb.T @ (G_E . ohr)
                mE = singles.tile([MC, LOC], f32)
                nc.vector.tensor_mul(mE[:], psGE[0:MC, 0:LOC], ohr_sb[:, :])
                mR = singles.tile([MC, LOC], f32)
                nc.vector.tensor_mul(mR[:], psGR[0:MC, 0:LOC], ohr_sb[:, :])

                with tc.tile_pool(name="psE", bufs=1, space="PSUM") as psEp:
                    aps = psEp.tile([1, 1024], f32)
                    pps = psEp.tile([1, 1024], f32)
                    nc.tensor.matmul(aps[:, 0:H0], wa_sb[:], mR[:, 0:H0])
                    nc.tensor.matmul(aps[:, H0:LOC], wa_sb[:], mR[:, H0:LOC])
                    nc.tensor.matmul(pps[:, 0:H0], wb_sb[:], mE[:, 0:H0])
                    nc.tensor.matmul(pps[:, H0:LOC], wb_sb[:], mE[:, H0:LOC])

                    bsb = epsp.tile([1, LOC], f32, tag="eps")
                    nc.vector.tensor_copy(bsb[:], pps[:, 0:LOC])
                    tsb = epsp.tile([1, LOC], f32, tag="eps")
                    nc.vector.tensor_mul(tsb[:], aps[:, 0:LOC], bsb[:])
                    lsb = epsp.tile([1, LOC], f32, tag="eps")
                    nc.scalar.activation(lsb[:], tsb[:], AF.Ln, bias=1.0)
                    lm = epsp.tile([1, LOC], f32, tag="eps")
                    nc.vector.tensor_mul(lm[:], lsb[:], vloc_sb[:])
                    part = singles.tile([1, 1], f32)
                    nc.vector.reduce_sum(part[:], lm[:], axis=mybir.AxisListType.X)
                    nc.sync.dma_start(outp[:, :], part[:])

    nc.compile()
    return nc


def _get_nc():
    global _BUILT
    if _BUILT is None:
        _BUILT = _build()
    return _BUILT


def _prep_inputs(input_f, target):
    bf16 = ml_dtypes.bfloat16
    x = np.ascontiguousarray(input_f.reshape(N_REAL, D), dtype=np.float32)
    lab = np.zeros(NP_PAD, dtype=np.int64)
    lab[:N_REAL] = target.reshape(-1)
    xp = np.ones((NP_PAD, D), dtype=np.float32)
    xp[:N_REAL] = x
    v = (lab != 0).astype(np.float32)

    # ohv[p, t, c]: j = 128*t + p; c<95: valid one-hot of class c+1; c=95: valid
    ohv = np.zeros((NP_PAD, MC), dtype=np.float32)
    idx = np.nonzero(v)[0]
    ohv[idx, lab[idx] - 1] = 1.0
    ohv[:, NCLS] = v
    ohv = ohv.reshape(JT, 128, MC).transpose(1, 0, 2)  # -> [p, t, c]
    ohv = np.ascontiguousarray(ohv).astype(bf16)

    wa = np.ones((MC, 1), dtype=np.float32)
    wa[NCLS, 0] = 0.0
    wb = np.full((MC, 1), -1.0, dtype=np.float32)
    wb[NCLS, 0] = 1.0

    in_maps = []
    for c in range(NCORES):
        rows = slice(c * LOC, (c + 1) * LOC)
        xa = np.concatenate(
            [xp[rows].reshape(LT, 128, D), xp.reshape(JT, 128, D)], axis=0
        )
        lab_loc = lab[rows]
        ohr = np.zeros((MC, LOC), dtype=np.float32)
        iloc = np.nonzero(lab_loc)[0]
        ohr[lab_loc[iloc] - 1, iloc] = 1.0
        ohr[NCLS, :] = 1.0
        in_maps.append(
            {
                "xa": np.ascontiguousarray(xa),
                "ohv": ohv,
                "ohr": ohr,
                "vloc": np.ascontiguousarray(v[rows][None, :]),
                "wa": wa,
                "wb": wb,
            }
        )
    n_valid = float(v.sum())
    return in_maps, n_valid


def kernel(input_f, target):
    from concourse.bass_utils import run_bass_kernel_spmd

    nc = _get_nc()
    in_maps, n_valid = _prep_inputs(input_f, target)
    res = run_bass_kernel_spmd(nc, in_maps, core_ids=list(range(NCORES)))
    total = sum(float(r["out"][0, 0]) for r in res.results)
    return np.float32(total / n_valid)


# revision 12
# speedup vs baseline: 1.0385x; 1.0385x over previous
"""Trainium2 Bass kernel for nn_ContrastLoss3 (multi-positive contrastive loss).

Math (matches the reference):
  f = L2-normalize(input_f.reshape(N, D)) rows;  sim = f @ f.T  (N = B*T = 6912)
  per valid row i:  A_i = sum_{j valid, lab_j == lab_i} exp(-sim_ij)
                    B_i = sum_{j valid, lab_j != lab_i} exp(+sim_ij)
                    loss_i = log(1 + A_i * B_i)
  out = sum_i valid loss_i / n_valid

Device strategy (8 cores, row-sharded):
  - Pad N to 7168 = 56*128. Each core owns a 896-row block (i), computes the
    full sim column-block S[j, i] for all j via PE matmuls (bf16, fp32 PSUM),
    contracting D=1024 in 8 k-tiles of 128.
  - E = exp(S), R = exp(-S) on the scalar engine (ACT), bf16.
  - Masked sums become matmuls against a one-hot label matrix OHV [j, 96]
    (95 classes x valid + a valid column), accumulated in PSUM across all 56
    j-tiles:  G_E[c, i] = sum_j ohv[j,c] E[j,i], same for R.
  - Epilogue: A_i = sum_c ohr[c,i] G_R[c,i]; B_i = G_E[95,i] - sum_c ohr G_E;
    loss_i = ln(1 + A_i B_i); masked row-sum -> one scalar per core.
  - Normalization: per-row sum-of-squares via ACT Square+accum in row layout,
    1/sqrt via a Newton iteration on DVE (seed 1/32 since q ~= D), scale+cast
    to bf16 on DVE, then on-device DMA xbar transpose to the [d, j] layout
    the PE needs.
  Host only pads/reshapes/transposes-none, builds one-hot index encodings,
  and sums the 8 per-core partials / n_valid.
"""

import numpy as np
import ml_dtypes

# ---- geometry (hardcoded for this problem) ----
B, T, D = 256, 27, 1024
N_REAL = B * T            # 6912
NP_PAD = 7168             # 56 * 128
NCORES = 8
LOC = NP_PAD // NCORES    # 896 rows per core
JT = NP_PAD // 128        # 56 column tiles
LT = LOC // 128           # 7 local tiles
TN = JT + LT              # 63 row tiles streamed in (local first)
KT = D // 128             # 8 contraction tiles
NCLS = 95                 # labels are 1..95
MC = 96                   # ohv cols: 95 classes + valid column
H0, H1 = 512, 384         # matmul free-dim split of 896 (PSUM bank = 512 fp32)

_BUILT = None


def _build(jt_count=JT, phase_a=True):
    import concourse.tile as tile
    from concourse import bacc, mybir

    f32 = mybir.dt.float32
    bf16 = mybir.dt.bfloat16
    AF = mybir.ActivationFunctionType
    OP = mybir.AluOpType

    nc = bacc.Bacc("TRN2", target_bir_lowering=False, debug=False)

    xa = nc.dram_tensor("xa", [TN, 128, D], f32, kind="ExternalInput")
    ohv = nc.dram_tensor("ohv", [128, JT, MC], bf16, kind="ExternalInput")
    ohr = nc.dram_tensor("ohr", [MC, LOC], f32, kind="ExternalInput")
    vloc = nc.dram_tensor("vloc", [1, LOC], f32, kind="ExternalInput")
    # reduction weight columns: wa = [1]*95+[0], wb = [-1]*95+[1]
    wa = nc.dram_tensor("wa", [MC, 1], f32, kind="ExternalInput")
    wb = nc.dram_tensor("wb", [MC, 1], f32, kind="ExternalInput")
    outp = nc.dram_tensor("out", [1, 1], f32, kind="ExternalOutput")

    with tile.TileContext(nc) as tc:
        with (
            tc.tile_pool(name="singles", bufs=1) as singles,
            tc.tile_pool(name="x32", bufs=3) as x32p,
            tc.tile_pool(name="xraw", bufs=10) as xrawp,
            tc.tile_pool(name="xn", bufs=3) as xnp_,
            tc.tile_pool(name="sq", bufs=2) as sqp,
            tc.tile_pool(name="newt", bufs=2) as ntp,
            tc.tile_pool(name="ebuf", bufs=2) as ep,
            tc.tile_pool(name="rbuf", bufs=2) as rp,
            tc.tile_pool(name="eps", bufs=2) as epsp,
        ):
            fT = singles.tile([128, KT, NP_PAD], bf16)     # [d_lo, d_hi, j]
            fLoc = singles.tile([128, KT, LOC], bf16)      # [d_lo, d_hi, i]
            ohv_sb = singles.tile([128, JT, MC], bf16)
            ohr_sb = singles.tile([MC, LOC], f32)
            vloc_sb = singles.tile([1, LOC], f32)
            sumsq = singles.tile([128, TN + 1], f32)
            s_all = singles.tile([128, TN + 1], f32)
            wa_sb = singles.tile([MC, 1], f32)
            wb_sb = singles.tile([MC, 1], f32)

            nc.sync.dma_start(ohv_sb[:], ohv[:, :, :])
            nc.sync.dma_start(ohr_sb[:], ohr[:, :])
            nc.sync.dma_start(vloc_sb[:], vloc[:, :])
            nc.sync.dma_start(wa_sb[:], wa[:, :])
            nc.sync.dma_start(wb_sb[:], wb[:, :])

            # ---- phase A: load, sumsq, rsqrt, normalize+cast, transpose ----
            # groups: tiles [0,7) = local block, then 7 groups of 8 full tiles
            groups = [list(range(0, LT))] + [
                list(range(LT + 8 * g, LT + min(8 * g + 8, JT))) for g in range(7)
            ]
            if not phase_a:
                groups = groups[:1]
            c0 = 1.0 / 32.0  # rsqrt seed: q ~= D = 1024

            for grp in groups:
                xraws = {}
                for t in grp:
                    x32 = x32p.tile([128, D], f32)
                    nc.sync.dma_start(x32[:], xa[t, :, :])
                    xraw = xrawp.tile([128, D], bf16)
                    nc.vector.tensor_scalar_mul(xraw[:], x32[:], 1.0)
                    sq = sqp.tile([128, D], bf16)
                    nc.scalar.activation(
                        sq[:], xraw[:], AF.Square,
                        accum_out=sumsq[:, t : t + 1],
                    )
                    xraws[t] = xraw
                a, b = grp[0], grp[-1] + 1
                q = sumsq[:, a:b]
                y = s_all[:, a:b]
                # y0 = c0*(1.5 - 0.5*c0^2*q) ; then Newton y <- y*(1.5 - 0.5*q*y^2)
                nc.vector.tensor_scalar(
                    out=y, in0=q, scalar1=-0.5 * c0**3, scalar2=1.5 * c0,
                    op0=OP.mult, op1=OP.add,
                )
                for _ in range(4):
                    t2 = ntp.tile([128, 8], f32)
                    w = b - a
                    nc.vector.tensor_mul(t2[:, :w], y, y)
                    nc.vector.tensor_mul(t2[:, :w], t2[:, :w], q)
                    nc.vector.tensor_scalar(
                        out=t2[:, :w], in0=t2[:, :w], scalar1=-0.5, scalar2=1.5,
                        op0=OP.mult, op1=OP.add,
                    )
                    nc.vector.tensor_mul(y, y, t2[:, :w])
                for t in grp:
                    xn = xnp_.tile([128, D], bf16)
                    nc.vector.tensor_scalar_mul(xn[:], xraws[t][:], s_all[:, t : t + 1])
                    if t < LT:
                        dest = fLoc[:, :, 128 * t : 128 * (t + 1)]
                    else:
                        tt = t - LT
                        dest = fT[:, :, 128 * tt : 128 * (tt + 1)]
                    nc.sync.dma_start_transpose(dest, xn[:])

            # ---- phases B+C: matmuls, exp, one-hot reductions, epilogue ----
            with tc.tile_pool(name="psG", bufs=1, space="PSUM") as psGp:
                psGE = psGp.tile([MC, 1024], f32)
                psGR = psGp.tile([MC, 1024], f32)

                with tc.tile_pool(name="psS", bufs=2, space="PSUM") as psSp:
                    for jt in range(jt_count):
                        psS = psSp.tile([128, 1024], f32)
                        for k in range(KT):
                            w = fT[:, k, 128 * jt : 128 * (jt + 1)]
                            nc.tensor.matmul(
                                psS[:, 0:H0], w, fLoc[:, k, 0:H0],
                                start=(k == 0), stop=(k == KT - 1),
                            )
                            nc.tensor.matmul(
                                psS[:, H0:LOC], w, fLoc[:, k, H0:LOC],
                                start=(k == 0), stop=(k == KT - 1),
                            )
                        e_t = ep.tile([128, LOC], bf16)
                        nc.scalar.activation(e_t[:], psS[:, 0:LOC], AF.Exp, scale=1.0)
                        r_t = rp.tile([128, LOC], bf16)
                        nc.scalar.activation(r_t[:], psS[:, 0:LOC], AF.Exp, scale=-1.0)
                        w2 = ohv_sb[:, jt, :]
                        st, sp = (jt == 0), (jt == jt_count - 1)
                        nc.tensor.matmul(psGE[:, 0:H0], w2, e_t[:, 0:H0],
                                         start=st, stop=sp, skip_group_check=True)
                        nc.tensor.matmul(psGE[:, H0:LOC], w2, e_t[:, H0:LOC],
                                         start=st, stop=sp, skip_group_check=True)
                        nc.tensor.matmul(psGR[:, 0:H0], w2, r_t[:, 0:H0],
                                         start=st, stop=sp, skip_group_check=True)
                        nc.tensor.matmul(psGR[:, H0:LOC], w2, r_t[:, H0:LOC],
                                         start=st, stop=sp, skip_group_check=True)

                # ---- epilogue ----
                # ohr rows 0..94: one-hot of local labels; row 95: all ones.
                # A = wa.T @ (G_R . ohr);  B = wb.T @ (G_E . ohr)
                mE = singles.tile([MC, LOC], f32)
                nc.vector.tensor_mul(mE[:], psGE[0:MC, 0:LOC], ohr_sb[:, :])
                mR = singles.tile([MC, LOC], f32)
                nc.vector.tensor_mul(mR[:], psGR[0:MC, 0:LOC], ohr_sb[:, :])

                with tc.tile_pool(name="psE", bufs=1, space="PSUM") as psEp:
                    aps = psEp.tile([1, 1024], f32)
                    pps = psEp.tile([1, 1024], f32)
                    nc.tensor.matmul(aps[:, 0:H0], wa_sb[:], mR[:, 0:H0])
                    nc.tensor.matmul(aps[:, H0:LOC], wa_sb[:], mR[:, H0:LOC])
                    nc.tensor.matmul(pps[:, 0:H0], wb_sb[:], mE[:, 0:H0])
                    nc.tensor.matmul(pps[:, H0:LOC], wb_sb[:], mE[:, H0:LOC])

                    bsb = epsp.tile([1, LOC], f32, tag="eps")
                    nc.vector.tensor_copy(bsb[:], pps[:, 0:LOC])
                    tsb = epsp.tile([1, LOC], f32, tag="eps")
                    nc.vector.tensor_mul(tsb[:], aps[:, 0:LOC], bsb[:])
                    lsb = epsp.tile([1, LOC], f32, tag="eps")
                    nc.scalar.activation(lsb[:], tsb[:], AF.Ln, bias=1.0)
                    lm = epsp.tile([1, LOC], f32, tag="eps")
                    nc.vector.tensor_mul(lm[:], lsb[:], vloc_sb[:])
                    part = singles.tile([1, 1], f32)
                    nc.vector.reduce_sum(part[:], lm[:], axis=mybir.AxisListType.X)
                    nc.sync.dma_start(outp[:, :], part[:])

    nc.compile()
    return nc


def _get_nc():
    global _BUILT
    if _BUILT is None:
        _BUILT = _build()
    return _BUILT


def _prep_inputs(input_f, target):
    bf16 = ml_dtypes.bfloat16
    x = np.ascontiguousarray(input_f.reshape(N_REAL, D), dtype=np.float32)
    lab = np.zeros(NP_PAD, dtype=np.int64)
    lab[:N_REAL] = target.reshape(-1)
    xp = np.ones((NP_PAD, D), dtype=np.float32)
    xp[:N_REAL] = x
    v = (lab != 0).astype(np.float32)

    # ohv[p, t, c]: j = 128*t + p; c<95: valid one-hot of class c+1; c=95: valid
    ohv = np.zeros((NP_PAD, MC), dtype=np.float32)
    idx = np.nonzero(v)[0]
    ohv[idx, lab[idx] - 1] = 1.0
    ohv[:, NCLS] = v
    ohv = ohv.reshape(JT, 128, MC).transpose(1, 0, 2)  # -> [p, t, c]
    ohv = np.ascontiguousarray(ohv).astype(bf16)

    wa = np.ones((MC, 1), dtype=np.float32)
    wa[NCLS, 0] = 0.0
    wb = np.full((MC, 1), -1.0, dtype=np.float32)
    wb[NCLS, 0] = 1.0

    in_maps = []
    for c in range(NCORES):
        rows = slice(c * LOC, (c + 1) * LOC)
        xa = np.concatenate(
            [xp[rows].reshape(LT, 128, D), xp.reshape(JT, 128, D)], axis=0
        )
        lab_loc = lab[rows]
        ohr = np.zeros((MC, LOC), dtype=np.float32)
        iloc = np.nonzero(lab_loc)[0]
        ohr[lab_loc[iloc] - 1, iloc] = 1.0
        ohr[NCLS, :] = 1.0
        in_maps.append(
            {
                "xa": np.ascontiguousarray(xa),
                "ohv": ohv,
                "ohr": ohr,
                "vloc": np.ascontiguousarray(v[rows][None, :]),
                "wa": wa,
                "wb": wb,
            }
        )
    n_valid = float(v.sum())
    return in_maps, n_valid


def kernel(input_f, target):
    from concourse.bass_utils import run_bass_kernel_spmd

    nc = _get_nc()
    in_maps, n_valid = _prep_inputs(input_f, target)
    res = run_bass_kernel_spmd(nc, in_maps, core_ids=list(range(NCORES)))
    total = sum(float(r["out"][0, 0]) for r in res.results)
    return np.float32(total / n_valid)
